# revision 10
# baseline (speedup 1.0000x reference)
"""Trainium2 Bass kernel for nn_Atten (Restormer-style transposed attention).

Shapes (hardcoded): pre/cur [8, 16384, 128] f32.  8 NeuronCores, one batch
sample per core (fully independent data parallelism).

Per-core pipeline, channels-on-partitions [c=128, hw] layout:
  1. LN stats+apply in natural [hw_tile=128, c] layout (bn_stats), cast bf16,
     TensorE-transpose into a zero-padded [c, (h+2)*(w+4)] image buffer.
  2. conv1x1 + depthwise 3x3 folded: 9 accumulated TensorE matmuls per output
     chunk, tap t using lhsT_t[cin,o] = W1'[o,cin]*w2[o,t] and a rhs offset of
     dy*WP+dx into the padded buffer (zero pads give exact conv padding).
  3. l2norm row sums via ScalarE Square+accum_out; attention logits by
     streaming 128x128 transposes of q,k back to [hw,c] and accumulating
     lhsT=kT,rhs=qT matmuls into one PSUM bank; 1/||q||,1/||k|| applied as
     per-partition scalars around one more transpose; softmax on-chip.
  4. v produced per 4-row chunk (same folded conv), att@v + out-conv + final
     transpose-back, residual added from a fresh DMA of cur, DMA out.
"""

import os
import sys

sys.path.insert(0, "/opt/trn_rl_repo")

import numpy as np
import ml_dtypes

import concourse.bass as bass
import concourse.tile as tile
from concourse import bacc
from concourse import mybir
from concourse.bass_utils import run_bass_kernel_spmd
from concourse.masks import make_identity

BF16 = mybir.dt.bfloat16
F32 = mybir.dt.float32

C = 128
H = W = 128
HW = H * W
WP = W + 4          # padded row stride (2 left, 2 right) keeps 4B alignment
HP = H + 2          # one pad row top and bottom
BASE = WP + 2       # flat index of image pixel (0,0)
FLAT = HP * WP      # 17160
FLAT_ALLOC = FLAT + 8
NCHUNK_B = (WP * H) // 512  # 33 (WP*H = 16896 = 33*512)

# tap order t = (dy+1)*3 + (dx+1)
TAP_DELTAS = [dy * WP + dx for dy in (-1, 0, 1) for dx in (-1, 0, 1)]

_TOP = {0, 1, 2}
_BOT = {6, 7, 8}
_LEFT = {0, 3, 6}
_RIGHT = {2, 5, 8}
REGION_MISSING = [
    _TOP, _BOT, _LEFT, _RIGHT,
    _TOP | _LEFT, _TOP | _RIGHT, _BOT | _LEFT, _BOT | _RIGHT,
]

_CACHE = {}


def _sub(t, off, dims):
    """AP at element offset `off` into tile t's free space with free `dims`."""
    a = t[...]
    return bass.AP(tensor=a.tensor, offset=a.offset + off,
                   ap=[list(a.ap[0])] + [list(d) for d in dims])


def _build_nc():
    nc = bacc.Bacc()

    pre_d = nc.dram_tensor("pre", [HW, C], F32, kind="ExternalInput")
    cur_d = nc.dram_tensor("cur", [HW, C], F32, kind="ExternalInput")
    wq_d = nc.dram_tensor("wq", [C, 9, C], BF16, kind="ExternalInput")
    wk_d = nc.dram_tensor("wk", [C, 9, C], BF16, kind="ExternalInput")
    wv_d = nc.dram_tensor("wv", [C, 9, C], BF16, kind="ExternalInput")
    wo_d = nc.dram_tensor("wo", [C, C], BF16, kind="ExternalInput")
    beq_d = nc.dram_tensor("beq", [C, 1], F32, kind="ExternalInput")
    bek_d = nc.dram_tensor("bek", [C, 1], F32, kind="ExternalInput")
    bev_d = nc.dram_tensor("bev", [C, 1], F32, kind="ExternalInput")
    bcq_d = nc.dram_tensor("bcq", [C, 8], F32, kind="ExternalInput")
    bck_d = nc.dram_tensor("bck", [C, 8], F32, kind="ExternalInput")
    bcv_d = nc.dram_tensor("bcv", [C, 8], F32, kind="ExternalInput")
    ob_d = nc.dram_tensor("ob", [C, 1], F32, kind="ExternalInput")
    out_d = nc.dram_tensor("out", [HW, C], F32, kind="ExternalOutput")

    pre_r = pre_d[:, :].rearrange("(y x) c -> x y c", x=W)
    cur_r = cur_d[:, :].rearrange("(y x) c -> x y c", x=W)
    out_r = out_d[:, :].rearrange("(y x) c -> x y c", x=W)

    dram = {"pre": pre_r, "cur": cur_r, "out": out_r,
            "wq": wq_d, "wk": wk_d, "wv": wv_d, "wo": wo_d,
            "beq": beq_d, "bek": bek_d, "bev": bev_d,
            "bcq": bcq_d, "bck": bck_d, "bcv": bcv_d, "ob": ob_d}
    with tile.TileContext(nc) as tc:
        _emit(nc, tc, dram)
    nc.finalize()
    return nc


def _emit(nc, tc, dram):
    AX = mybir.AxisListType
    OP = mybir.AluOpType
    AF = mybir.ActivationFunctionType

    with tc.tile_pool(name="persist", bufs=1) as P:
        ident = P.tile([128, 128], BF16)
        make_identity(nc, ident)

        wq = P.tile([C, 9, C], BF16)
        wk = P.tile([C, 9, C], BF16)
        wv = P.tile([C, 9, C], BF16)
        wo = P.tile([C, C], BF16)
        beq = P.tile([C, 1], F32)
        bek = P.tile([C, 1], F32)
        bev = P.tile([C, 1], F32)
        bcq = P.tile([C, 8], F32)
        bck = P.tile([C, 8], F32)
        bcv = P.tile([C, 8], F32)
        ob = P.tile([C, 1], F32)
        for sb, name in ((wq, "wq"), (wk, "wk"), (wv, "wv"), (wo, "wo"),
                         (beq, "beq"), (bek, "bek"), (bev, "bev"),
                         (bcq, "bcq"), (bck, "bck"), (bcv, "bcv"),
                         (ob, "ob")):
            nc.sync.dma_start(out=sb[...], in_=dram[name][...])

        cur_ln = P.tile([128, FLAT_ALLOC], BF16)
        pre_ln = P.tile([128, FLAT_ALLOC], BF16)
        q_pad = P.tile([128, FLAT_ALLOC], BF16)
        k_pad = P.tile([128, FLAT_ALLOC], BF16)
        nc.gpsimd.memset(cur_ln[...], 0.0)
        nc.gpsimd.memset(pre_ln[...], 0.0)

        eps1 = P.tile([128, 1], F32)
        nc.vector.memset(eps1[...], 1e-5)
        zero1 = P.tile([128, 1], F32)
        nc.vector.memset(zero1[...], 0.0)
        rsq = P.tile([128, 1], F32)
        rsk = P.tile([128, 1], F32)
        attT = P.tile([128, 128], BF16)

        # ---------------- stage A: LN + transpose ----------------
        def stage_a_group(g, src_r, dst_ln, apool, spool, ppool):
            x4 = apool.tile([128, 4, C], F32, tag="x4")
            for yy in range(4):
                nc.gpsimd.dma_start(out=x4[:, yy, :],
                                    in_=src_r[:, 4 * g + yy, :])
            tp = ppool.tile([128, 4, 128], BF16, tag="tp")
            for yy in range(4):
                st6 = spool.tile([128, 6], F32, tag="st6")
                nc.vector.bn_stats(out=st6[...], in_=x4[:, yy, :])
                mv = spool.tile([128, 2], F32, tag="mv")
                nc.vector.bn_aggr(out=mv[...], in_=st6[...])
                rstd = spool.tile([128, 1], F32, tag="rstd")
                nc.scalar.activation(out=rstd[...], in_=mv[:, 1:2],
                                     func=AF.Sqrt, bias=eps1[...], scale=1.0)
                nc.vector.reciprocal(out=rstd[...], in_=rstd[...])
                xln = spool.tile([128, C], BF16, tag="xln")
                nc.vector.tensor_scalar(out=xln[...], in0=x4[:, yy, :],
                                        scalar1=mv[:, 0:1], scalar2=rstd[...],
                                        op0=OP.subtract, op1=OP.mult)
                nc.tensor.transpose(tp[:, yy, :], xln[...], ident[...])
            dst = _sub(dst_ln, BASE + WP * 4 * g, [[WP, 4], [1, W]])
            nc.vector.tensor_copy(out=dst, in_=tp[...])

        def stage_b_chunk(i, src_ln, w_taps, b_eff, dst, bpool, use_act):
            s = BASE + 512 * i
            ps = bpool.tile([128, 512], F32, tag="ps")
            for t in range(9):
                rhs = _sub(src_ln, s + TAP_DELTAS[t], [[1, 512]])
                nc.tensor.matmul(ps[...], w_taps[:, t, :], rhs,
                                 start=(t == 0), stop=(t == 8))
            d = _sub(dst, s, [[1, 512]])
            if use_act:
                nc.scalar.activation(out=d, in_=ps[...], func=AF.Identity,
                                     bias=b_eff[...], scale=1.0)
            else:
                nc.vector.tensor_scalar_add(out=d, in0=ps[...],
                                            scalar1=b_eff[...])

        def border_fix(dst, bc):
            regs = [
                (BASE + 1, [[1, 126]]),
                (BASE + WP * (H - 1) + 1, [[1, 126]]),
                (BASE + WP, [[WP, 126]]),
                (BASE + WP + (W - 1), [[WP, 126]]),
                (BASE, [[1, 1]]),
                (BASE + (W - 1), [[1, 1]]),
                (BASE + WP * (H - 1), [[1, 1]]),
                (BASE + WP * (H - 1) + (W - 1), [[1, 1]]),
            ]
            for r, (off, dims) in enumerate(regs):
                v = _sub(dst, off, dims)
                nc.vector.tensor_scalar_add(out=v, in0=v,
                                            scalar1=bc[:, r:r + 1])

        # A(cur)
        with tc.tile_pool(name="a1", bufs=3) as apool, \
                tc.tile_pool(name="as1", bufs=4) as spool, \
                tc.tile_pool(name="ap1", bufs=3, space="PSUM") as ppool:
            for g in range(H // 4):
                stage_a_group(g, dram["cur"], cur_ln, apool, spool, ppool)

        # B(q) interleaved with A(pre)
        with tc.tile_pool(name="a2", bufs=3) as apool, \
                tc.tile_pool(name="as2", bufs=4) as spool, \
                tc.tile_pool(name="ap2", bufs=2, space="PSUM") as ppool, \
                tc.tile_pool(name="bp1", bufs=3, space="PSUM") as bpool:
            for i in range(NCHUNK_B):
                stage_b_chunk(i, cur_ln, wq, beq, q_pad, bpool, use_act=True)
                if i < H // 4:
                    stage_a_group(i, dram["pre"], pre_ln, apool, spool, ppool)

        # B(k), border fixes, l2 norms
        with tc.tile_pool(name="bp2", bufs=3, space="PSUM") as bpool, \
                tc.tile_pool(name="np", bufs=2) as npool:
            for i in range(NCHUNK_B):
                stage_b_chunk(i, pre_ln, wk, bek, k_pad, bpool, use_act=False)
            border_fix(q_pad, bcq)
            border_fix(k_pad, bck)
            for src, rs in ((q_pad, rsq), (k_pad, rsk)):
                parts = npool.tile([128, 8], F32, tag="parts")
                for j in range(8):
                    sq = npool.tile([128, 16, W], BF16, tag="sq")
                    view = _sub(src, BASE + WP * 16 * j, [[WP, 16], [1, W]])
                    nc.scalar.activation(out=sq[...], in_=view,
                                         func=AF.Square, bias=zero1[...],
                                         accum_out=parts[:, j:j + 1])
                ss = npool.tile([128, 1], F32, tag="ss")
                nc.vector.reduce_sum(out=ss[...], in_=parts[...], axis=AX.X)
                nc.scalar.activation(out=rs[...], in_=ss[...], func=AF.Sqrt,
                                     bias=zero1[...], scale=1.0)
                nc.vector.reciprocal(out=rs[...], in_=rs[...])

        # C: attention logits + softmax
        with tc.tile_pool(name="cq", bufs=3) as cpool, \
                tc.tile_pool(name="cp", bufs=2, space="PSUM") as cppool, \
                tc.tile_pool(name="attp", bufs=1, space="PSUM") as attp, \
                tc.tile_pool(name="smx", bufs=1) as smx, \
                tc.tile_pool(name="smp", bufs=1, space="PSUM") as smp:
            att_ps = attp.tile([128, 128], F32)
            for g in range(H // 4):
                tq = cppool.tile([128, 4, 128], BF16, tag="tq")
                tk = cppool.tile([128, 4, 128], BF16, tag="tk")
                for yy in range(4):
                    y = 4 * g + yy
                    nc.tensor.transpose(
                        tq[:, yy, :],
                        _sub(q_pad, BASE + WP * y, [[1, W]]), ident[...])
                    nc.tensor.transpose(
                        tk[:, yy, :],
                        _sub(k_pad, BASE + WP * y, [[1, W]]), ident[...])
                qT = cpool.tile([128, 4, 128], BF16, tag="qT")
                kT = cpool.tile([128, 4, 128], BF16, tag="kT")
                nc.vector.tensor_copy(out=qT[...], in_=tq[...])
                nc.vector.tensor_copy(out=kT[...], in_=tk[...])
                for yy in range(4):
                    nc.tensor.matmul(att_ps[...], kT[:, yy, :], qT[:, yy, :],
                                     start=(g == 0 and yy == 0),
                                     stop=(g == H // 4 - 1 and yy == 3),
                                     skip_group_check=True)

            attT_sc = smx.tile([128, 128], BF16)
            nc.vector.tensor_scalar_mul(out=attT_sc[...], in0=att_ps[...],
                                        scalar1=rsk[...])
            at2 = smp.tile([128, 128], F32, tag="at2")
            nc.tensor.matmul(at2[...], attT_sc[...], ident[...],
                             start=True, stop=True)
            logits = smx.tile([128, 128], F32)
            nc.vector.tensor_scalar_mul(out=logits[...], in0=at2[...],
                                        scalar1=rsq[...])
            mx = smx.tile([128, 1], F32)
            nc.vector.reduce_max(out=mx[...], in_=logits[...], axis=AX.X)
            nmx = smx.tile([128, 1], F32)
            nc.vector.tensor_scalar_mul(out=nmx[...], in0=mx[...],
                                        scalar1=-1.0)
            pexp = smx.tile([128, 128], BF16)
            sume = smx.tile([128, 1], F32)
            nc.scalar.activation(out=pexp[...], in_=logits[...], func=AF.Exp,
                                 bias=nmx[...], scale=1.0,
                                 accum_out=sume[...])
            rsum = smx.tile([128, 1], F32)
            nc.vector.reciprocal(out=rsum[...], in_=sume[...])
            att_bf = smx.tile([128, 128], BF16)
            nc.vector.tensor_scalar_mul(out=att_bf[...], in0=pexp[...],
                                        scalar1=rsum[...])
            atp = smp.tile([128, 128], BF16, tag="atp")
            nc.tensor.transpose(atp[...], att_bf[...], ident[...])
            nc.vector.tensor_copy(out=attT[...], in_=atp[...])

        # D: v, att@v, out conv, residual, store
        with tc.tile_pool(name="dd", bufs=3) as dpool, \
                tc.tile_pool(name="dp", bufs=2, space="PSUM") as dppool:
            for g in range(H // 4):
                y0 = 4 * g
                vbuf = dpool.tile([128, 4, W], BF16, tag="vbuf")
                for h in range(2):
                    s = BASE + WP * (y0 + 2 * h)
                    pv = dppool.tile([128, 264], F32, tag="pv")
                    for t in range(9):
                        rhs = _sub(pre_ln, s + TAP_DELTAS[t], [[1, 264]])
                        nc.tensor.matmul(pv[...], wv[:, t, :], rhs,
                                         start=(t == 0), stop=(t == 8))
                    src = _sub(pv, 0, [[WP, 2], [1, W]])
                    nc.vector.tensor_scalar_add(
                        out=vbuf[:, 2 * h:2 * h + 2, :], in0=src,
                        scalar1=bev[...])
                for r, (off, dims) in _v_regions(y0):
                    vv = _sub(vbuf, off, dims)
                    nc.vector.tensor_scalar_add(out=vv, in0=vv,
                                                scalar1=bcv[:, r:r + 1])
                pav = dppool.tile([128, 512], F32, tag="pav")
                nc.tensor.matmul(pav[...], attT[...], vbuf[...],
                                 start=True, stop=True)
                av = dpool.tile([128, 512], BF16, tag="av")
                nc.scalar.activation(out=av[...], in_=pav[...], func=AF.Copy,
                                     bias=0.0, scale=1.0)
                poc = dppool.tile([128, 512], F32, tag="poc")
                nc.tensor.matmul(poc[...], wo[...], av[...],
                                 start=True, stop=True)
                oc = dpool.tile([128, 4, W], BF16, tag="oc")
                nc.scalar.activation(out=oc[...],
                                     in_=_sub(poc, 0, [[W, 4], [1, W]]),
                                     func=AF.Identity, bias=ob[...], scale=1.0)
                po = dppool.tile([128, 4, 128], BF16, tag="po")
                for yy in range(4):
                    nc.tensor.transpose(po[:, yy, :], oc[:, yy, :],
                                        ident[...])
                res = dpool.tile([128, 4, C], F32, tag="res")
                nc.gpsimd.dma_start(out=res[...],
                                    in_=dram["cur"][:, y0:y0 + 4, :])
                osb = dpool.tile([128, 4, C], F32, tag="osb")
                nc.vector.tensor_tensor(out=osb[...], in0=po[...],
                                        in1=res[...], op=OP.add)
                nc.gpsimd.dma_start(out=dram["out"][:, y0:y0 + 4, :],
                                    in_=osb[...])


def _v_regions(y0):
    out = []
    rows = [y for y in range(y0, y0 + 4) if 1 <= y <= H - 2]
    if rows:
        first = rows[0] - y0
        n = len(rows)
        out.append((2, (first * W + 0, [[W, n], [1, 1]])))
        out.append((3, (first * W + (W - 1), [[W, n], [1, 1]])))
    if y0 == 0:
        out.append((0, (1, [[1, 126]])))
        out.append((4, (0, [[1, 1]])))
        out.append((5, (W - 1, [[1, 1]])))
    if y0 + 4 == H:
        base = 3 * W
        out.append((1, (base + 1, [[1, 126]])))
        out.append((6, (base, [[1, 1]])))
        out.append((7, (base + W - 1, [[1, 1]])))
    return out


def _prep_weights(inputs):
    f = np.float32
    ln1_w = inputs["ln1_w"].astype(f)
    ln1_b = inputs["ln1_b"].astype(f)
    ln2_w = inputs["ln2_w"].astype(f)
    ln2_b = inputs["ln2_b"].astype(f)
    q_w1 = inputs["q_w1"].astype(f)
    q_b1 = inputs["q_b1"].astype(f)
    q_w2 = inputs["q_w2"].astype(f).reshape(C, 9)
    q_b2 = inputs["q_b2"].astype(f)
    kv_w1 = inputs["kv_w1"].astype(f)
    kv_b1 = inputs["kv_b1"].astype(f)
    kv_w2 = inputs["kv_w2"].astype(f).reshape(2 * C, 9)
    kv_b2 = inputs["kv_b2"].astype(f)
    out_w = inputs["out_w"].astype(f)
    out_b = inputs["out_b"].astype(f)

    bf = ml_dtypes.bfloat16

    def fold(w1, b1, lnw, lnb, w2, b2):
        w1p = w1 * lnw[None, :]                      # [o, cin]
        b1p = b1 + w1 @ lnb                          # [o]
        lhs = w1p.T[:, None, :] * w2.T[None, :, :]   # [cin, 9, o]
        beff = b2 + b1p * w2.sum(axis=1)             # [o]
        bc = np.stack([-(w2[:, sorted(m)].sum(axis=1)) * b1p
                       for m in REGION_MISSING], axis=1)  # [o, 8]
        return lhs.astype(bf), beff.astype(f), bc.astype(f)

    wq, beq, bcq = fold(q_w1, q_b1, ln2_w, ln2_b, q_w2, q_b2)
    wk, bek, bck = fold(kv_w1[:C], kv_b1[:C], ln1_w, ln1_b,
                        kv_w2[:C], kv_b2[:C])
    wv, bev, bcv = fold(kv_w1[C:], kv_b1[C:], ln1_w, ln1_b,
                        kv_w2[C:], kv_b2[C:])
    return {
        "wq": np.ascontiguousarray(wq),
        "wk": np.ascontiguousarray(wk),
        "wv": np.ascontiguousarray(wv),
        "wo": np.ascontiguousarray(out_w.T.astype(bf)),
        "beq": beq.reshape(C, 1), "bek": bek.reshape(C, 1),
        "bev": bev.reshape(C, 1),
        "bcq": np.ascontiguousarray(bcq), "bck": np.ascontiguousarray(bck),
        "bcv": np.ascontiguousarray(bcv),
        "ob": out_b.reshape(C, 1).astype(f),
    }


def kernel(**inputs):
    if "nc" not in _CACHE:
        _CACHE["nc"] = _build_nc()
    nc = _CACHE["nc"]

    wmap = _prep_weights(inputs)
    pre = np.asarray(inputs["pre"], dtype=np.float32)
    cur = np.asarray(inputs["cur"], dtype=np.float32)
    in_maps = []
    for s in range(8):
        m = {"pre": np.ascontiguousarray(pre[s]),
             "cur": np.ascontiguousarray(cur[s])}
        m.update(wmap)
        in_maps.append(m)

    trace = bool(os.environ.get("BASS_KERNEL_TRACE"))
    try:
        res = run_bass_kernel_spmd(nc, in_maps, core_ids=list(range(8)),
                                   trace=trace)
    except ModuleNotFoundError:
        res = run_bass_kernel_spmd(nc, in_maps, core_ids=list(range(8)),
                                   trace=False)
    if trace and getattr(res, "exec_time_ns", None) is not None:
        print(f"HW exec time: {res.exec_time_ns} ns")
        _CACHE["exec_time_ns"] = res.exec_time_ns
    out = np.stack([r["out"] for r in res.results], axis=0)
    return out


# revision 15
# speedup vs baseline: 1.0450x; 1.0450x over previous
"""Trainium2 Bass kernel for nn_Atten (Restormer-style transposed attention).

Shapes (hardcoded): pre/cur [8, 16384, 128] f32.  8 NeuronCores, one batch
sample per core (fully independent data parallelism).

Per-core pipeline, channels-on-partitions [c=128, hw] layout:
  1. LN stats+apply in natural [hw_tile=128, c] layout (bn_stats), cast bf16,
     TensorE-transpose into a zero-padded [c, (h+2)*(w+4)] image buffer.
  2. conv1x1 + depthwise 3x3 folded: 9 accumulated TensorE matmuls per output
     chunk, tap t using lhsT_t[cin,o] = W1'[o,cin]*w2[o,t] and a rhs offset of
     dy*WP+dx into the padded buffer (zero pads give exact conv padding).
  3. l2norm row sums via ScalarE Square+accum_out; attention logits by
     streaming 128x128 transposes of q,k back to [hw,c] and accumulating
     lhsT=kT,rhs=qT matmuls into one PSUM bank; 1/||q||,1/||k|| applied as
     per-partition scalars around one more transpose; softmax on-chip.
  4. v produced per 4-row chunk (same folded conv), att@v + out-conv + final
     transpose-back, residual added from a fresh DMA of cur, DMA out.
"""

import os
import sys

sys.path.insert(0, "/opt/trn_rl_repo")

import numpy as np
import ml_dtypes

import concourse.bass as bass
import concourse.tile as tile
from concourse import bacc
from concourse import mybir
from concourse.bass_utils import run_bass_kernel_spmd
from concourse.masks import make_identity

BF16 = mybir.dt.bfloat16
F32 = mybir.dt.float32

C = 128
H = W = 128
HW = H * W
WP = W + 4          # padded row stride (2 left, 2 right) keeps 4B alignment
HP = H + 2          # one pad row top and bottom
BASE = WP + 2       # flat index of image pixel (0,0)
FLAT = HP * WP      # 17160
FLAT_ALLOC = FLAT + 8
NCHUNK_B = (WP * H) // 512  # 33 (WP*H = 16896 = 33*512)

# tap order t = (dy+1)*3 + (dx+1)
TAP_DELTAS = [dy * WP + dx for dy in (-1, 0, 1) for dx in (-1, 0, 1)]

_TOP = {0, 1, 2}
_BOT = {6, 7, 8}
_LEFT = {0, 3, 6}
_RIGHT = {2, 5, 8}
REGION_MISSING = [
    _TOP, _BOT, _LEFT, _RIGHT,
    _TOP | _LEFT, _TOP | _RIGHT, _BOT | _LEFT, _BOT | _RIGHT,
]

_CACHE = {}


def _sub(t, off, dims):
    """AP at element offset `off` into tile t's free space with free `dims`."""
    a = t[...]
    return bass.AP(tensor=a.tensor, offset=a.offset + off,
                   ap=[list(a.ap[0])] + [list(d) for d in dims])


def _build_nc():
    nc = bacc.Bacc()

    pre_d = nc.dram_tensor("pre", [HW, C], F32, kind="ExternalInput")
    cur_d = nc.dram_tensor("cur", [HW, C], F32, kind="ExternalInput")
    wq_d = nc.dram_tensor("wq", [C, 9, C], BF16, kind="ExternalInput")
    wk_d = nc.dram_tensor("wk", [C, 9, C], BF16, kind="ExternalInput")
    wv_d = nc.dram_tensor("wv", [C, 9, C], BF16, kind="ExternalInput")
    wo_d = nc.dram_tensor("wo", [C, C], BF16, kind="ExternalInput")
    beq_d = nc.dram_tensor("beq", [C, 1], F32, kind="ExternalInput")
    bek_d = nc.dram_tensor("bek", [C, 1], F32, kind="ExternalInput")
    bev_d = nc.dram_tensor("bev", [C, 1], F32, kind="ExternalInput")
    bcq_d = nc.dram_tensor("bcq", [C, 8], F32, kind="ExternalInput")
    bck_d = nc.dram_tensor("bck", [C, 8], F32, kind="ExternalInput")
    bcv_d = nc.dram_tensor("bcv", [C, 8], F32, kind="ExternalInput")
    ob_d = nc.dram_tensor("ob", [C, 1], F32, kind="ExternalInput")
    out_d = nc.dram_tensor("out", [HW, C], F32, kind="ExternalOutput")

    pre_r = pre_d[:, :].rearrange("(y x) c -> x y c", x=W)
    cur_r = cur_d[:, :].rearrange("(y x) c -> x y c", x=W)
    out_r = out_d[:, :].rearrange("(y x) c -> x y c", x=W)

    dram = {"pre": pre_r, "cur": cur_r, "out": out_r,
            "wq": wq_d, "wk": wk_d, "wv": wv_d, "wo": wo_d,
            "beq": beq_d, "bek": bek_d, "bev": bev_d,
            "bcq": bcq_d, "bck": bck_d, "bcv": bcv_d, "ob": ob_d}
    with tile.TileContext(nc) as tc:
        _emit(nc, tc, dram)
    nc.finalize()
    return nc


def _emit(nc, tc, dram):
    AX = mybir.AxisListType
    OP = mybir.AluOpType
    AF = mybir.ActivationFunctionType

    with tc.tile_pool(name="persist", bufs=1) as P:
        ident = P.tile([128, 128], BF16)
        make_identity(nc, ident)

        wq = P.tile([C, 9, C], BF16)
        wk = P.tile([C, 9, C], BF16)
        wv = P.tile([C, 9, C], BF16)
        wo = P.tile([C, C], BF16)
        beq = P.tile([C, 1], F32)
        bek = P.tile([C, 1], F32)
        bev = P.tile([C, 1], F32)
        bcq = P.tile([C, 8], F32)
        bck = P.tile([C, 8], F32)
        bcv = P.tile([C, 8], F32)
        ob = P.tile([C, 1], F32)
        for sb, name in ((wq, "wq"), (wk, "wk"), (wv, "wv"), (wo, "wo"),
                         (beq, "beq"), (bek, "bek"), (bev, "bev"),
                         (bcq, "bcq"), (bck, "bck"), (bcv, "bcv"),
                         (ob, "ob")):
            nc.sync.dma_start(out=sb[...], in_=dram[name][...])

        cur_ln = P.tile([128, FLAT_ALLOC], BF16)
        pre_ln = P.tile([128, FLAT_ALLOC], BF16)
        q_pad = P.tile([128, FLAT_ALLOC], BF16)
        k_pad = P.tile([128, FLAT_ALLOC], BF16)
        nc.gpsimd.memset(cur_ln[...], 0.0)
        nc.gpsimd.memset(pre_ln[...], 0.0)

        eps1 = P.tile([128, 1], F32)
        nc.vector.memset(eps1[...], 1e-5)
        zero1 = P.tile([128, 1], F32)
        nc.vector.memset(zero1[...], 0.0)
        rsq = P.tile([128, 1], F32)
        rsk = P.tile([128, 1], F32)
        attT = P.tile([128, 128], BF16)

        # ---------------- stage A: LN + transpose ----------------
        def stage_a_group(g, src_r, dst_ln, apool, spool, ppool):
            x4 = apool.tile([128, 4, C], F32, tag="x4")
            nc.sync.dma_start(out=x4[...], in_=src_r[:, 4 * g:4 * g + 4, :])
            tp = ppool.tile([128, 4, 128], BF16, tag="tp")
            for yy in range(4):
                st6 = spool.tile([128, 6], F32, tag="st6")
                nc.vector.bn_stats(out=st6[...], in_=x4[:, yy, :])
                mv = spool.tile([128, 2], F32, tag="mv")
                nc.vector.bn_aggr(out=mv[...], in_=st6[...])
                rstd = spool.tile([128, 1], F32, tag="rstd")
                nc.scalar.activation(out=rstd[...], in_=mv[:, 1:2],
                                     func=AF.Sqrt, bias=eps1[...], scale=1.0)
                nc.vector.reciprocal(out=rstd[...], in_=rstd[...])
                xln = spool.tile([128, C], BF16, tag="xln")
                nc.vector.tensor_scalar(out=xln[...], in0=x4[:, yy, :],
                                        scalar1=mv[:, 0:1], scalar2=rstd[...],
                                        op0=OP.subtract, op1=OP.mult)
                nc.tensor.transpose(tp[:, yy, :], xln[...], ident[...])
            dst = _sub(dst_ln, BASE + WP * 4 * g, [[WP, 4], [1, W]])
            nc.vector.tensor_copy(out=dst, in_=tp[...])

        def stage_b_chunk(i, src_ln, w_taps, b_eff, dst, bpool, use_act):
            s = BASE + 512 * i
            ps = bpool.tile([128, 512], F32, tag="ps")
            for t in range(9):
                rhs = _sub(src_ln, s + TAP_DELTAS[t], [[1, 512]])
                nc.tensor.matmul(ps[...], w_taps[:, t, :], rhs,
                                 start=(t == 0), stop=(t == 8))
            d = _sub(dst, s, [[1, 512]])
            if use_act:
                nc.scalar.activation(out=d, in_=ps[...], func=AF.Identity,
                                     bias=b_eff[...], scale=1.0)
            else:
                nc.vector.tensor_scalar_add(out=d, in0=ps[...],
                                            scalar1=b_eff[...])

        def border_fix(dst, bc):
            regs = [
                (BASE + 1, [[1, 126]]),
                (BASE + WP * (H - 1) + 1, [[1, 126]]),
                (BASE + WP, [[WP, 126]]),
                (BASE + WP + (W - 1), [[WP, 126]]),
                (BASE, [[1, 1]]),
                (BASE + (W - 1), [[1, 1]]),
                (BASE + WP * (H - 1), [[1, 1]]),
                (BASE + WP * (H - 1) + (W - 1), [[1, 1]]),
            ]
            for r, (off, dims) in enumerate(regs):
                v = _sub(dst, off, dims)
                nc.vector.tensor_scalar_add(out=v, in0=v,
                                            scalar1=bc[:, r:r + 1])

        # A(cur)
        with tc.tile_pool(name="a1", bufs=3) as apool, \
                tc.tile_pool(name="as1", bufs=4) as spool, \
                tc.tile_pool(name="ap1", bufs=3, space="PSUM") as ppool:
            for g in range(H // 4):
                stage_a_group(g, dram["cur"], cur_ln, apool, spool, ppool)

        # B(q) interleaved with A(pre)
        with tc.tile_pool(name="a2", bufs=3) as apool, \
                tc.tile_pool(name="as2", bufs=4) as spool, \
                tc.tile_pool(name="ap2", bufs=2, space="PSUM") as ppool, \
                tc.tile_pool(name="bp1", bufs=4, space="PSUM") as bpool:
            for i in range(NCHUNK_B):
                stage_b_chunk(i, cur_ln, wq, beq, q_pad, bpool, use_act=True)
                if i < H // 4:
                    stage_a_group(i, dram["pre"], pre_ln, apool, spool, ppool)

        # B(k), border fixes, l2 norms
        with tc.tile_pool(name="bp2", bufs=4, space="PSUM") as bpool, \
                tc.tile_pool(name="np", bufs=2) as npool:
            for i in range(NCHUNK_B):
                stage_b_chunk(i, pre_ln, wk, bek, k_pad, bpool, use_act=True)
            border_fix(q_pad, bcq)
            border_fix(k_pad, bck)
            for src, rs in ((q_pad, rsq), (k_pad, rsk)):
                parts = npool.tile([128, 8], F32, tag="parts")
                for j in range(8):
                    sq = npool.tile([128, 16, W], BF16, tag="sq")
                    view = _sub(src, BASE + WP * 16 * j, [[WP, 16], [1, W]])
                    nc.scalar.activation(out=sq[...], in_=view,
                                         func=AF.Square, bias=zero1[...],
                                         accum_out=parts[:, j:j + 1])
                ss = npool.tile([128, 1], F32, tag="ss")
                nc.vector.reduce_sum(out=ss[...], in_=parts[...], axis=AX.X)
                nc.scalar.activation(out=rs[...], in_=ss[...], func=AF.Sqrt,
                                     bias=zero1[...], scale=1.0)
                nc.vector.reciprocal(out=rs[...], in_=rs[...])

        # C: attention logits + softmax
        with tc.tile_pool(name="cq", bufs=3) as cpool, \
                tc.tile_pool(name="cp", bufs=2, space="PSUM") as cppool, \
                tc.tile_pool(name="attp", bufs=1, space="PSUM") as attp, \
                tc.tile_pool(name="smx", bufs=1) as smx, \
                tc.tile_pool(name="smp", bufs=1, space="PSUM") as smp:
            att_ps = attp.tile([128, 128], F32)
            for g in range(H // 4):
                tq = cppool.tile([128, 4, 128], BF16, tag="tq")
                tk = cppool.tile([128, 4, 128], BF16, tag="tk")
                for yy in range(4):
                    y = 4 * g + yy
                    nc.tensor.transpose(
                        tq[:, yy, :],
                        _sub(q_pad, BASE + WP * y, [[1, W]]), ident[...])
                    nc.tensor.transpose(
                        tk[:, yy, :],
                        _sub(k_pad, BASE + WP * y, [[1, W]]), ident[...])
                qT = cpool.tile([128, 4, 128], BF16, tag="qT")
                kT = cpool.tile([128, 4, 128], BF16, tag="kT")
                nc.vector.tensor_copy(out=qT[...], in_=tq[...])
                nc.scalar.activation(out=kT[...], in_=tk[...],
                                     func=AF.Copy, bias=0.0, scale=1.0)
                for yy in range(4):
                    nc.tensor.matmul(att_ps[...], kT[:, yy, :], qT[:, yy, :],
                                     start=(g == 0 and yy == 0),
                                     stop=(g == H // 4 - 1 and yy == 3),
                                     skip_group_check=True)

            attT_sc = smx.tile([128, 128], BF16)
            nc.vector.tensor_scalar_mul(out=attT_sc[...], in0=att_ps[...],
                                        scalar1=rsk[...])
            at2 = smp.tile([128, 128], F32, tag="at2")
            nc.tensor.matmul(at2[...], attT_sc[...], ident[...],
                             start=True, stop=True)
            logits = smx.tile([128, 128], F32)
            nc.vector.tensor_scalar_mul(out=logits[...], in0=at2[...],
                                        scalar1=rsq[...])
            mx = smx.tile([128, 1], F32)
            nc.vector.reduce_max(out=mx[...], in_=logits[...], axis=AX.X)
            nmx = smx.tile([128, 1], F32)
            nc.vector.tensor_scalar_mul(out=nmx[...], in0=mx[...],
                                        scalar1=-1.0)
            pexp = smx.tile([128, 128], BF16)
            sume = smx.tile([128, 1], F32)
            nc.scalar.activation(out=pexp[...], in_=logits[...], func=AF.Exp,
                                 bias=nmx[...], scale=1.0,
                                 accum_out=sume[...])
            rsum = smx.tile([128, 1], F32)
            nc.vector.reciprocal(out=rsum[...], in_=sume[...])
            att_bf = smx.tile([128, 128], BF16)
            nc.vector.tensor_scalar_mul(out=att_bf[...], in0=pexp[...],
                                        scalar1=rsum[...])
            atp = smp.tile([128, 128], BF16, tag="atp")
            nc.tensor.transpose(atp[...], att_bf[...], ident[...])
            nc.vector.tensor_copy(out=attT[...], in_=atp[...])

        # D: v, att@v, out conv, residual, store
        with tc.tile_pool(name="dd", bufs=3) as dpool, \
                tc.tile_pool(name="dp", bufs=2, space="PSUM") as dppool:
            for g in range(H // 4):
                y0 = 4 * g
                vbuf = dpool.tile([128, 4, W], BF16, tag="vbuf")
                for h in range(2):
                    s = BASE + WP * (y0 + 2 * h)
                    pv = dppool.tile([128, 264], F32, tag="pv")
                    for t in range(9):
                        rhs = _sub(pre_ln, s + TAP_DELTAS[t], [[1, 264]])
                        nc.tensor.matmul(pv[...], wv[:, t, :], rhs,
                                         start=(t == 0), stop=(t == 8))
                    src = _sub(pv, 0, [[WP, 2], [1, W]])
                    nc.vector.tensor_scalar_add(
                        out=vbuf[:, 2 * h:2 * h + 2, :], in0=src,
                        scalar1=bev[...])
                for r, (off, dims) in _v_regions(y0):
                    vv = _sub(vbuf, off, dims)
                    nc.vector.tensor_scalar_add(out=vv, in0=vv,
                                                scalar1=bcv[:, r:r + 1])
                pav = dppool.tile([128, 512], F32, tag="pav")
                nc.tensor.matmul(pav[...], attT[...], vbuf[...],
                                 start=True, stop=True)
                av = dpool.tile([128, 512], BF16, tag="av")
                nc.scalar.activation(out=av[...], in_=pav[...], func=AF.Copy,
                                     bias=0.0, scale=1.0)
                poc = dppool.tile([128, 512], F32, tag="poc")
                nc.tensor.matmul(poc[...], wo[...], av[...],
                                 start=True, stop=True)
                oc = dpool.tile([128, 4, W], BF16, tag="oc")
                nc.scalar.activation(out=oc[...],
                                     in_=_sub(poc, 0, [[W, 4], [1, W]]),
                                     func=AF.Identity, bias=ob[...], scale=1.0)
                po = dppool.tile([128, 4, 128], BF16, tag="po")
                for yy in range(4):
                    nc.tensor.transpose(po[:, yy, :], oc[:, yy, :],
                                        ident[...])
                res = dpool.tile([128, 4, C], F32, tag="res")
                nc.sync.dma_start(out=res[...],
                                  in_=dram["cur"][:, y0:y0 + 4, :])
                osb = dpool.tile([128, 4, C], F32, tag="osb")
                nc.vector.tensor_tensor(out=osb[...], in0=po[...],
                                        in1=res[...], op=OP.add)
                nc.sync.dma_start(out=dram["out"][:, y0:y0 + 4, :],
                                  in_=osb[...])


def _v_regions(y0):
    out = []
    rows = [y for y in range(y0, y0 + 4) if 1 <= y <= H - 2]
    if rows:
        first = rows[0] - y0
        n = len(rows)
        out.append((2, (first * W + 0, [[W, n], [1, 1]])))
        out.append((3, (first * W + (W - 1), [[W, n], [1, 1]])))
    if y0 == 0:
        out.append((0, (1, [[1, 126]])))
        out.append((4, (0, [[1, 1]])))
        out.append((5, (W - 1, [[1, 1]])))
    if y0 + 4 == H:
        base = 3 * W
        out.append((1, (base + 1, [[1, 126]])))
        out.append((6, (base, [[1, 1]])))
        out.append((7, (base + W - 1, [[1, 1]])))
    return out


def _prep_weights(inputs):
    f = np.float32
    ln1_w = inputs["ln1_w"].astype(f)
    ln1_b = inputs["ln1_b"].astype(f)
    ln2_w = inputs["ln2_w"].astype(f)
    ln2_b = inputs["ln2_b"].astype(f)
    q_w1 = inputs["q_w1"].astype(f)
    q_b1 = inputs["q_b1"].astype(f)
    q_w2 = inputs["q_w2"].astype(f).reshape(C, 9)
    q_b2 = inputs["q_b2"].astype(f)
    kv_w1 = inputs["kv_w1"].astype(f)
    kv_b1 = inputs["kv_b1"].astype(f)
    kv_w2 = inputs["kv_w2"].astype(f).reshape(2 * C, 9)
    kv_b2 = inputs["kv_b2"].astype(f)
    out_w = inputs["out_w"].astype(f)
    out_b = inputs["out_b"].astype(f)

    bf = ml_dtypes.bfloat16

    def fold(w1, b1, lnw, lnb, w2, b2):
        w1p = w1 * lnw[None, :]                      # [o, cin]
        b1p = b1 + w1 @ lnb                          # [o]
        lhs = w1p.T[:, None, :] * w2.T[None, :, :]   # [cin, 9, o]
        beff = b2 + b1p * w2.sum(axis=1)             # [o]
        bc = np.stack([-(w2[:, sorted(m)].sum(axis=1)) * b1p
                       for m in REGION_MISSING], axis=1)  # [o, 8]
        return lhs.astype(bf), beff.astype(f), bc.astype(f)

    wq, beq, bcq = fold(q_w1, q_b1, ln2_w, ln2_b, q_w2, q_b2)
    wk, bek, bck = fold(kv_w1[:C], kv_b1[:C], ln1_w, ln1_b,
                        kv_w2[:C], kv_b2[:C])
    wv, bev, bcv = fold(kv_w1[C:], kv_b1[C:], ln1_w, ln1_b,
                        kv_w2[C:], kv_b2[C:])
    return {
        "wq": np.ascontiguousarray(wq),
        "wk": np.ascontiguousarray(wk),
        "wv": np.ascontiguousarray(wv),
        "wo": np.ascontiguousarray(out_w.T.astype(bf)),
        "beq": beq.reshape(C, 1), "bek": bek.reshape(C, 1),
        "bev": bev.reshape(C, 1),
        "bcq": np.ascontiguousarray(bcq), "bck": np.ascontiguousarray(bck),
        "bcv": np.ascontiguousarray(bcv),
        "ob": out_b.reshape(C, 1).astype(f),
    }


def kernel(**inputs):
    if "nc" not in _CACHE:
        _CACHE["nc"] = _build_nc()
    nc = _CACHE["nc"]

    wmap = _prep_weights(inputs)
    pre = np.asarray(inputs["pre"], dtype=np.float32)
    cur = np.asarray(inputs["cur"], dtype=np.float32)
    in_maps = []
    for s in range(8):
        m = {"pre": np.ascontiguousarray(pre[s]),
             "cur": np.ascontiguousarray(cur[s])}
        m.update(wmap)
        in_maps.append(m)

    trace = bool(os.environ.get("BASS_KERNEL_TRACE"))
    try:
        res = run_bass_kernel_spmd(nc, in_maps, core_ids=list(range(8)),
                                   trace=trace)
    except ModuleNotFoundError:
        res = run_bass_kernel_spmd(nc, in_maps, core_ids=list(range(8)),
                                   trace=False)
    if trace and getattr(res, "exec_time_ns", None) is not None:
        print(f"HW exec time: {res.exec_time_ns} ns")
        _CACHE["exec_time_ns"] = res.exec_time_ns
    out = np.stack([r["out"] for r in res.results], axis=0)
    return out
